# revision 14
# baseline (speedup 1.0000x reference)
"""Fused transformer block (rmsnorm+causal attention+rmsnorm+squared-relu MLP)
for one TRN2 chip (8 NeuronCores), SPMD via bass/Tile.

Sharding: core = 2*b + half for batch b.
  - Attention/QKV: head-parallel — each core computes q,k,v and attention for
    8 of the 16 heads over the full T=2048 sequence of its batch (uniform
    causal structure across cores -> single SPMD program).
  - After attention, a pairwise AllToAll within {2b, 2b+1} swaps per-head
    outputs so each core holds all 16 heads for half the tokens; proj, the
    residual, rmsnorm2 and the MLP then run token-parallel (1024 tokens/core).
Matmul inputs are bf16 (fp32 PSUM accumulation); residual/stats paths stay
fp32. rmsnorm scales are folded into the QKV/fc1 weights host-side, and the
1/sqrt(head_dim) factor into the Q weights.
"""

import sys

sys.path.insert(0, "/opt/trn_rl_repo")

import numpy as np
import ml_dtypes

import concourse.bass as bass
import concourse.mybir as mybir
import concourse.tile as tile
from concourse import bacc
from concourse import bass_utils
from concourse.masks import make_identity

BF = mybir.dt.bfloat16
F32 = mybir.dt.float32
AF = mybir.ActivationFunctionType

B, T, C = 4, 2048, 1024
H, D = 16, 64
HPC = 8  # heads per core
TQ = 1024  # tokens per core after the exchange
EPS = 1e-6
NCORES = 8

_cache = {}


def _build(collective: bool = True, num_devices: int = NCORES):
    nc = bacc.Bacc(
        "TRN2", target_bir_lowering=False, debug=False, num_devices=num_devices
    )
    xT = nc.dram_tensor("xT", [C, T], F32, kind="ExternalInput").ap()
    xq = nc.dram_tensor("xq", [TQ, C], F32, kind="ExternalInput").ap()
    wqk = nc.dram_tensor("wqk", [C, 2 * HPC * D], BF, kind="ExternalInput").ap()
    wv = nc.dram_tensor("wv", [C, HPC * D], BF, kind="ExternalInput").ap()
    wp = nc.dram_tensor("wp", [C, C], BF, kind="ExternalInput").ap()
    wf1 = nc.dram_tensor("wf1", [32, 8, 128, 128], BF, kind="ExternalInput").ap()
    wf2 = nc.dram_tensor("wf2", [4 * C, C], BF, kind="ExternalInput").ap()
    tokoff = nc.dram_tensor("tokoff", [1, 1], mybir.dt.uint32, kind="ExternalInput").ap()
    out = nc.dram_tensor("out", [TQ, C], F32, kind="ExternalOutput").ap()

    global _g_tokoff
    _g_tokoff = tokoff
    with tile.TileContext(nc) as tc:
        _body(tc, xT, xq, wqk, wv, wp, wf1, wf2, out, collective)
    nc.compile()
    return nc


def _body(tc, xT, xq, wqk, wv, wp, wf1, wf2, out, collective):
    nc = tc.nc
    from contextlib import ExitStack

    ctx = ExitStack()
    const = ctx.enter_context(tc.tile_pool(name="const", bufs=1))
    ps = ctx.enter_context(tc.tile_pool(name="ps", bufs=5, space="PSUM"))
    psacc = ctx.enter_context(tc.tile_pool(name="psacc", bufs=2, space="PSUM"))
    psden = ctx.enter_context(tc.tile_pool(name="psden", bufs=1, space="PSUM"))
    dram = ctx.enter_context(tc.tile_pool(name="dram", bufs=1, space="DRAM"))

    # ---- constants ----
    ident = const.tile([128, 128], BF)
    make_identity(nc, ident)
    ones64 = const.tile([128, 64], BF)
    nc.vector.memset(ones64, 1.0)
    ones128 = const.tile([128, 128], BF)
    nc.vector.memset(ones128, 1.0)
    eps_sb = const.tile([128, 1], F32)
    nc.vector.memset(eps_sb, EPS)

    pool_y = tc.alloc_tile_pool(name="pool_y", bufs=1)
    pool_ab = tc.alloc_tile_pool(name="pool_ab", bufs=1)
    workA = tc.alloc_tile_pool(name="workA", bufs=2)
    workB = tc.alloc_tile_pool(name="workB", bufs=5)

    # ---- resident weights (phase A/B) ----
    wqk_sb = pool_ab.tile([128, 8, 1024], BF, tag="wqk_sb")
    nc.sync.dma_start(out=wqk_sb, in_=wqk.rearrange("(ci p) m -> p ci m", p=128))
    wv_sb = pool_ab.tile([128, 8, 512], BF, tag="wv_sb")
    nc.sync.dma_start(out=wv_sb, in_=wv.rearrange("(ci p) m -> p ci m", p=128))

    QT = pool_ab.tile([128, 4, T], BF, tag="QT")  # per head-pair [2*64, t]
    KT = pool_ab.tile([128, 4, T], BF, tag="KT")
    V = pool_ab.tile([128, 16, 512], BF, tag="V")  # [t-tile, 8 heads * 64]
    yT = pool_y.tile([128, 4, T], BF, tag="yT")  # attention out per pair

    # =============== Phase A: rmsnorm1 + QKV (chunks of 512 tokens) ========
    xTr = xT.rearrange("(ci p) t -> p ci t", p=128)
    for tcx in range(4):
        tsl = slice(tcx * 512, (tcx + 1) * 512)
        xTc = workA.tile([128, 8, 512], F32, tag="xTc")
        for ci in range(8):
            nc.sync.dma_start(out=xTc[:, ci, :], in_=xTr[:, ci, tsl])
        # x^2 (bf16) -> PE partition-reduce -> rsqrt -> scale
        x2b = workA.tile([128, 8, 512], BF, tag="x2b")
        for ci in range(8):
            nc.vector.tensor_mul(x2b[:, ci, :], xTc[:, ci, :], xTc[:, ci, :])
        rb_ps = ps.tile([128, 512], F32, tag="mm")
        for ci in range(8):
            nc.tensor.matmul(
                rb_ps[:], ones128[:], x2b[:, ci, :],
                start=(ci == 0), stop=(ci == 7),
            )
        sq = workA.tile([128, 512], F32, tag="sq")
        nc.scalar.activation(sq, rb_ps, AF.Sqrt, bias=eps_sb[:], scale=1.0 / C)
        rb = workA.tile([128, 512], F32, tag="rb")
        nc.vector.reciprocal(rb, sq)
        xnc = workA.tile([128, 8, 512], BF, tag="xnc")
        for ci in range(8):
            nc.vector.tensor_mul(xnc[:, ci, :], xTc[:, ci, :], rb)

        # Q^T/K^T columns for this chunk
        for m in range(8):
            qk_ps = ps.tile([128, 512], F32, tag="mm")
            for ci in range(8):
                nc.tensor.matmul(
                    qk_ps[:],
                    wqk_sb[:, ci, m * 128 : (m + 1) * 128],
                    xnc[:, ci, :],
                    start=(ci == 0), stop=(ci == 7),
                )
            dst = QT[:, m, tsl] if m < 4 else KT[:, m - 4, tsl]
            nc.vector.tensor_copy(dst, qk_ps)
        # V rows for this chunk
        for tt in range(4):
            v_ps = ps.tile([128, 512], F32, tag="mm")
            for ci in range(8):
                nc.tensor.matmul(
                    v_ps[:],
                    xnc[:, ci, tt * 128 : (tt + 1) * 128],
                    wv_sb[:, ci, :],
                    start=(ci == 0), stop=(ci == 7),
                )
            nc.vector.tensor_copy(V[:, tcx * 4 + tt, :], v_ps)

    # =============== Phase B: attention per (head-pair, q-chunk) ===========
    for hp in range(4):
        for qc in range(4):
            nkt = 4 * qc + 4
            qsl = slice(qc * 512, (qc + 1) * 512)
            yav = psacc.tile([128, 512], F32, tag="acc_ps")
            acc_a = workB.tile([128, 512], BF, tag="acc_a")
            acc_b = workB.tile([128, 512], BF, tag="acc_b")
            for kt in range(nkt):
                ksl = slice(kt * 128, (kt + 1) * 128)
                sa = ps.tile([128, 512], F32, tag="mm")
                sb = ps.tile([128, 512], F32, tag="mm")
                nc.tensor.matmul(
                    sa[:], KT[0:64, hp, ksl], QT[0:64, hp, qsl],
                    start=True, stop=True,
                )
                nc.tensor.matmul(
                    sb[:], KT[64:128, hp, ksl], QT[64:128, hp, qsl],
                    start=True, stop=True,
                )
                ea = workB.tile([128, 512], BF, tag="ea")
                eb = workB.tile([128, 512], BF, tag="eb")
                nc.scalar.activation(ea, sa, AF.Exp)
                nc.scalar.activation(eb, sb, AF.Exp)
                if kt >= nkt - 4:
                    j = kt - (nkt - 4)
                    for e in (ea, eb):
                        nc.gpsimd.affine_select(
                            out=e, in_=e,
                            compare_op=mybir.AluOpType.is_ge,
                            fill=0.0, base=-128 * j,
                            channel_multiplier=-1, pattern=[[1, 512]],
                        )
                if kt == 0:
                    nc.vector.tensor_copy(acc_a, ea)
                    nc.vector.tensor_copy(acc_b, eb)
                else:
                    nc.vector.tensor_add(acc_a, acc_a, ea)
                    nc.vector.tensor_add(acc_b, acc_b, eb)
                nc.tensor.matmul(
                    yav[0:64, :], V[:, kt, 2 * hp * 64 : (2 * hp + 1) * 64], ea[:],
                    start=(kt == 0), stop=(kt == nkt - 1),
                    tile_position=(0, 0),
                )
                nc.tensor.matmul(
                    yav[64:128, :],
                    V[:, kt, (2 * hp + 1) * 64 : (2 * hp + 2) * 64], eb[:],
                    start=(kt == 0), stop=(kt == nkt - 1),
                    tile_position=(0, 64),
                )
            den = psden.tile([128, 512], F32, tag="den")
            nc.tensor.matmul(
                den[0:64, :], ones64[:], acc_a[:],
                start=True, stop=True, tile_position=(0, 0),
            )
            nc.tensor.matmul(
                den[64:128, :], ones64[:], acc_b[:],
                start=True, stop=True, tile_position=(0, 64),
            )
            rden = workB.tile([128, 512], F32, tag="rden")
            nc.vector.reciprocal(rden, den)
            nc.vector.tensor_mul(yT[:, hp, qsl], yav, rden)
    workB.release()
    workA.release()
    pool_ab.release()

    # =============== Phase C: pairwise head exchange =======================
    # AllGather the per-head attention outputs within each core pair, then
    # each core keeps its token half (dynamic column offset from `tokoff`).
    inb = dram.tile([512, T], BF, tag="inb")
    outb = dram.tile([1024, T], BF, tag="outb")
    for hp in range(4):
        nc.sync.dma_start(out=inb[hp * 128 : (hp + 1) * 128, :], in_=yT[:, hp, :])
    if collective:
        nc.gpsimd.collective_compute(
            "AllGather",
            mybir.AluOpType.bypass,
            replica_groups=[[0, 1], [2, 3], [4, 5], [6, 7]],
            ins=[inb[:].opt()],
            outs=[outb[:].opt()],
        )
    else:
        nc.sync.dma_start(out=outb[:512, :], in_=inb[:])
        nc.sync.dma_start(out=outb[512:, :], in_=inb[:])
    pool_y.release()

    tok_reg = nc.sync.alloc_register("tokoff_reg")
    nc.sync.reg_load(tok_reg, _g_tokoff[0:1, 0:1])
    tok0 = nc.sync.snap(tok_reg, donate=True, min_val=0, max_val=TQ)

    pool_d = tc.alloc_tile_pool(name="pool_d", bufs=1)
    pool_e = tc.alloc_tile_pool(name="pool_e", bufs=1)
    workD = tc.alloc_tile_pool(name="workD", bufs=2)
    yTf = pool_d.tile([128, 8, 1024], BF, tag="yTf")
    for r in range(8):
        nc.sync.dma_start(
            out=yTf[:, r, :],
            in_=outb[r * 128 : (r + 1) * 128, bass.ds(tok0, TQ)],
        )

    # =============== Phase D: proj + residual + rmsnorm2 + transpose =======
    wp_sb = pool_d.tile([128, 8, 1024], BF, tag="wp_sb")
    nc.sync.dma_start(out=wp_sb, in_=wp.rearrange("(ci p) m -> p ci m", p=128))
    xqr = xq.rearrange("(tt p) c -> p tt c", p=128)

    zd = dram.tile([TQ, C], F32, tag="zd")
    xn2 = pool_d.tile([128, 8, 1024], BF, tag="xn2")
    for tt in range(8):
        xq_tt = workD.tile([128, 1024], F32, tag="xq_tt")
        nc.sync.dma_start(out=xq_tt, in_=xqr[:, tt, :])
        z_tt = workD.tile([128, 1024], F32, tag="z_tt")
        for coh in range(2):
            csl = slice(coh * 512, (coh + 1) * 512)
            pp = psacc.tile([128, 512], F32, tag="acc_ps")
            for ci in range(8):
                nc.tensor.matmul(
                    pp[:],
                    yTf[:, ci, tt * 128 : (tt + 1) * 128],
                    wp_sb[:, ci, csl],
                    start=(ci == 0), stop=(ci == 7),
                )
            nc.vector.tensor_add(z_tt[:, csl], xq_tt[:, csl], pp)
        nc.sync.dma_start(out=zd[tt * 128 : (tt + 1) * 128, :], in_=z_tt)
        sq2 = workD.tile([128, 1024], F32, tag="sq2")
        ss2 = workD.tile([128, 1], F32, tag="ss2")
        nc.scalar.activation(sq2, z_tt, AF.Square, accum_out=ss2)
        s2 = workD.tile([128, 1], F32, tag="s2")
        nc.scalar.activation(s2, ss2, AF.Sqrt, bias=eps_sb[:], scale=1.0 / C)
        r2 = workD.tile([128, 1], F32, tag="r2")
        nc.vector.reciprocal(r2, s2)
        nc.vector.tensor_scalar_mul(xn2[:, tt, :], z_tt, r2)

    xn2T = pool_e.tile([128, 8, 1024], BF, tag="xn2T")
    for ci in range(8):
        for tt in range(8):
            tp = ps.tile([128, 128], BF, tag="mm")
            nc.tensor.transpose(tp[:], xn2[:, tt, ci * 128 : (ci + 1) * 128], ident[:])
            nc.any.tensor_copy(xn2T[:, ci, tt * 128 : (tt + 1) * 128], tp)
    workD.release()
    workE = tc.alloc_tile_pool(name="workE", bufs=3)
    poolH = tc.alloc_tile_pool(name="poolH", bufs=10)
    poolW2 = tc.alloc_tile_pool(name="poolW2", bufs=2)

    # =============== Phase E: MLP (fc1/fc2 group-pipelined, no staging) ====
    out_acc = pool_e.tile([128, 8, 1024], F32, tag="out_acc")
    wf2r = wf2.rearrange("(m p) co -> m p co", p=128)
    outr = out.rearrange("(tt p) c -> p tt c", p=128)
    for mg in range(4):
        hrows = []
        wf2g = poolW2.tile([128, 8, 1024], BF, tag="wf2g")
        for j in range(8):
            m = mg * 8 + j
            nc.sync.dma_start(out=wf2g[:, j, :], in_=wf2r[m])
            w1s = workE.tile([128, 8, 128], BF, tag="w1s")
            nc.sync.dma_start(out=w1s, in_=wf1[m].rearrange("ci p q -> p ci q"))
            hrow = poolH.tile([128, 1024], BF, tag="hrow")
            for tch in range(2):
                tsl = slice(tch * 512, (tch + 1) * 512)
                h_ps = ps.tile([128, 512], F32, tag="mm")
                for ci in range(8):
                    nc.tensor.matmul(
                        h_ps[:], w1s[:, ci, :], xn2T[:, ci, tsl],
                        start=(ci == 0), stop=(ci == 7),
                    )
                hrelu = workE.tile([128, 512], BF, tag="hrelu")
                nc.scalar.activation(hrelu, h_ps, AF.Relu)
                nc.vector.tensor_mul(hrow[:, tsl], hrelu, hrelu)
            hrows.append(hrow)
        for tt in range(8):
            for coh in range(2):
                csl = slice(coh * 512, (coh + 1) * 512)
                mp = psacc.tile([128, 512], F32, tag="acc_ps")
                for j in range(8):
                    nc.tensor.matmul(
                        mp[:],
                        hrows[j][:, tt * 128 : (tt + 1) * 128],
                        wf2g[:, j, csl],
                        start=(j == 0), stop=(j == 7),
                    )
                if mg == 0:
                    nc.vector.tensor_copy(out_acc[:, tt, csl], mp)
                else:
                    nc.vector.tensor_add(out_acc[:, tt, csl], out_acc[:, tt, csl], mp)
    for tt in range(8):
        for coh in range(2):
            csl = slice(coh * 512, (coh + 1) * 512)
            zr = workE.tile([128, 512], F32, tag="zr")
            nc.sync.dma_start(out=zr, in_=zd[tt * 128 : (tt + 1) * 128, csl])
            ot = workE.tile([128, 512], F32, tag="ot")
            nc.vector.tensor_add(ot, zr, out_acc[:, tt, csl])
            nc.sync.dma_start(out=outr[:, tt, csl], in_=ot)
    poolW2.release()
    poolH.release()
    workE.release()
    pool_e.release()
    pool_d.release()

    ctx.close()


def _prep_inputs(x, w_qkv, w_proj, w_fc1, w_fc2, scale1, scale2):
    """Host-side sharding: returns in_maps for the 8 cores."""
    bf = ml_dtypes.bfloat16
    Wq = (w_qkv[:C] * scale1[None, :]) * (1.0 / np.sqrt(D))
    Wk = w_qkv[C : 2 * C] * scale1[None, :]
    Wv = w_qkv[2 * C :] * scale1[None, :]
    wp_np = np.ascontiguousarray(w_proj.T).astype(bf)
    wf1T = (w_fc1 * scale2[None, :]).T  # [C, 4C]
    wf1_np = np.ascontiguousarray(
        wf1T.reshape(8, 128, 32, 128).transpose(2, 0, 1, 3)
    ).astype(bf)
    wf2_np = np.ascontiguousarray(w_fc2.T).astype(bf)

    in_maps = []
    for core in range(NCORES):
        b, half = divmod(core, 2)
        heads = [8 * half + j for j in range(HPC)]
        qk_rows = []
        for i in range(4):
            h0, h1 = heads[2 * i], heads[2 * i + 1]
            qk_rows.append(Wq[64 * h0 : 64 * h0 + 64])
            qk_rows.append(Wq[64 * h1 : 64 * h1 + 64])
        for i in range(4):
            h0, h1 = heads[2 * i], heads[2 * i + 1]
            qk_rows.append(Wk[64 * h0 : 64 * h0 + 64])
            qk_rows.append(Wk[64 * h1 : 64 * h1 + 64])
        wqk_np = np.ascontiguousarray(np.concatenate(qk_rows, axis=0).T).astype(bf)
        v_rows = np.concatenate(
            [Wv[64 * h : 64 * h + 64] for h in heads], axis=0
        )
        wv_np = np.ascontiguousarray(v_rows.T).astype(bf)
        in_maps.append(
            {
                "xT": np.ascontiguousarray(x[b].T).astype(np.float32),
                "xq": np.ascontiguousarray(
                    x[b, half * TQ : (half + 1) * TQ]
                ).astype(np.float32),
                "wqk": wqk_np,
                "wv": wv_np,
                "wp": wp_np,
                "wf1": wf1_np,
                "wf2": wf2_np,
                "tokoff": np.array([[half * TQ]], dtype=np.uint32),
            }
        )
    return in_maps


def get_nc(collective: bool = True):
    key = ("nc", collective)
    if key not in _cache:
        _cache[key] = _build(collective=collective)
    return _cache[key]


def kernel(x, w_qkv, w_proj, w_fc1, w_fc2, scale1, scale2):
    x = np.asarray(x, dtype=np.float32)
    w_qkv = np.asarray(w_qkv, dtype=np.float32)
    w_proj = np.asarray(w_proj, dtype=np.float32)
    w_fc1 = np.asarray(w_fc1, dtype=np.float32)
    w_fc2 = np.asarray(w_fc2, dtype=np.float32)
    scale1 = np.asarray(scale1, dtype=np.float32)
    scale2 = np.asarray(scale2, dtype=np.float32)

    nc = get_nc(collective=True)
    in_maps = _prep_inputs(x, w_qkv, w_proj, w_fc1, w_fc2, scale1, scale2)
    res = bass_utils.run_bass_kernel_spmd(
        nc, in_maps, core_ids=list(range(NCORES)), trace=False
    )
    out = np.empty((B, T, C), dtype=np.float32)
    for core in range(NCORES):
        b, half = divmod(core, 2)
        out[b, half * TQ : (half + 1) * TQ] = res.results[core]["out"]
    return out


# revision 15
# speedup vs baseline: 1.0016x; 1.0016x over previous
"""Fused transformer block (rmsnorm+causal attention+rmsnorm+squared-relu MLP)
for one TRN2 chip (8 NeuronCores), SPMD via bass/Tile.

Sharding: core = 2*b + half for batch b.
  - Attention/QKV: head-parallel — each core computes q,k,v and attention for
    8 of the 16 heads over the full T=2048 sequence of its batch (uniform
    causal structure across cores -> single SPMD program).
  - After attention, a pairwise AllToAll within {2b, 2b+1} swaps per-head
    outputs so each core holds all 16 heads for half the tokens; proj, the
    residual, rmsnorm2 and the MLP then run token-parallel (1024 tokens/core).
Matmul inputs are bf16 (fp32 PSUM accumulation); residual/stats paths stay
fp32. rmsnorm scales are folded into the QKV/fc1 weights host-side, and the
1/sqrt(head_dim) factor into the Q weights.
"""

import sys

sys.path.insert(0, "/opt/trn_rl_repo")

import numpy as np
import ml_dtypes

import concourse.bass as bass
import concourse.mybir as mybir
import concourse.tile as tile
from concourse import bacc
from concourse import bass_utils
from concourse.masks import make_identity

BF = mybir.dt.bfloat16
F32 = mybir.dt.float32
AF = mybir.ActivationFunctionType

B, T, C = 4, 2048, 1024
H, D = 16, 64
HPC = 8  # heads per core
TQ = 1024  # tokens per core after the exchange
EPS = 1e-6
NCORES = 8

_cache = {}


def _build(collective: bool = True, num_devices: int = NCORES):
    nc = bacc.Bacc(
        "TRN2", target_bir_lowering=False, debug=False, num_devices=num_devices
    )
    xT = nc.dram_tensor("xT", [C, T], F32, kind="ExternalInput").ap()
    xq = nc.dram_tensor("xq", [TQ, C], F32, kind="ExternalInput").ap()
    wqk = nc.dram_tensor("wqk", [C, 2 * HPC * D], BF, kind="ExternalInput").ap()
    wv = nc.dram_tensor("wv", [C, HPC * D], BF, kind="ExternalInput").ap()
    wp = nc.dram_tensor("wp", [C, C], BF, kind="ExternalInput").ap()
    wf1 = nc.dram_tensor("wf1", [32, 8, 128, 128], BF, kind="ExternalInput").ap()
    wf2 = nc.dram_tensor("wf2", [4 * C, C], BF, kind="ExternalInput").ap()
    tokoff = nc.dram_tensor("tokoff", [1, 1], mybir.dt.uint32, kind="ExternalInput").ap()
    out = nc.dram_tensor("out", [TQ, C], F32, kind="ExternalOutput").ap()

    global _g_tokoff
    _g_tokoff = tokoff
    with tile.TileContext(nc) as tc:
        _body(tc, xT, xq, wqk, wv, wp, wf1, wf2, out, collective)
    nc.compile()
    return nc


def _body(tc, xT, xq, wqk, wv, wp, wf1, wf2, out, collective):
    nc = tc.nc
    from contextlib import ExitStack

    ctx = ExitStack()
    const = ctx.enter_context(tc.tile_pool(name="const", bufs=1))
    ps = ctx.enter_context(tc.tile_pool(name="ps", bufs=5, space="PSUM"))
    psacc = ctx.enter_context(tc.tile_pool(name="psacc", bufs=3, space="PSUM"))
    dram = ctx.enter_context(tc.tile_pool(name="dram", bufs=1, space="DRAM"))

    # ---- constants ----
    ident = const.tile([128, 128], BF)
    make_identity(nc, ident)
    ones64 = const.tile([128, 64], BF)
    nc.vector.memset(ones64, 1.0)
    ones128 = const.tile([128, 128], BF)
    nc.vector.memset(ones128, 1.0)
    eps_sb = const.tile([128, 1], F32)
    nc.vector.memset(eps_sb, EPS)

    pool_y = tc.alloc_tile_pool(name="pool_y", bufs=1)
    pool_ab = tc.alloc_tile_pool(name="pool_ab", bufs=1)
    workA = tc.alloc_tile_pool(name="workA", bufs=2)
    workB = tc.alloc_tile_pool(name="workB", bufs=5)

    # ---- resident weights (phase A/B) ----
    wqk_sb = pool_ab.tile([128, 8, 1024], BF, tag="wqk_sb")
    nc.sync.dma_start(out=wqk_sb, in_=wqk.rearrange("(ci p) m -> p ci m", p=128))
    wv_sb = pool_ab.tile([128, 8, 512], BF, tag="wv_sb")
    nc.sync.dma_start(out=wv_sb, in_=wv.rearrange("(ci p) m -> p ci m", p=128))

    QT = pool_ab.tile([128, 4, T], BF, tag="QT")  # per head-pair [2*64, t]
    KT = pool_ab.tile([128, 4, T], BF, tag="KT")
    V = pool_ab.tile([128, 16, 512], BF, tag="V")  # [t-tile, 8 heads * 64]
    yT = pool_y.tile([128, 4, T], BF, tag="yT")  # attention out per pair

    # =============== Phase A: rmsnorm1 + QKV (chunks of 512 tokens) ========
    xTr = xT.rearrange("(ci p) t -> p ci t", p=128)
    for tcx in range(4):
        tsl = slice(tcx * 512, (tcx + 1) * 512)
        xTc = workA.tile([128, 8, 512], F32, tag="xTc")
        for ci in range(8):
            nc.sync.dma_start(out=xTc[:, ci, :], in_=xTr[:, ci, tsl])
        # x^2 (bf16) -> PE partition-reduce -> rsqrt -> scale
        x2b = workA.tile([128, 8, 512], BF, tag="x2b")
        for ci in range(8):
            nc.vector.tensor_mul(x2b[:, ci, :], xTc[:, ci, :], xTc[:, ci, :])
        rb_ps = ps.tile([128, 512], F32, tag="mm")
        for ci in range(8):
            nc.tensor.matmul(
                rb_ps[:], ones128[:], x2b[:, ci, :],
                start=(ci == 0), stop=(ci == 7),
            )
        sq = workA.tile([128, 512], F32, tag="sq")
        nc.scalar.activation(sq, rb_ps, AF.Sqrt, bias=eps_sb[:], scale=1.0 / C)
        rb = workA.tile([128, 512], F32, tag="rb")
        nc.vector.reciprocal(rb, sq)
        xnc = workA.tile([128, 8, 512], BF, tag="xnc")
        for ci in range(8):
            nc.vector.tensor_mul(xnc[:, ci, :], xTc[:, ci, :], rb)

        # Q^T/K^T columns for this chunk
        for m in range(8):
            qk_ps = ps.tile([128, 512], F32, tag="mm")
            for ci in range(8):
                nc.tensor.matmul(
                    qk_ps[:],
                    wqk_sb[:, ci, m * 128 : (m + 1) * 128],
                    xnc[:, ci, :],
                    start=(ci == 0), stop=(ci == 7),
                )
            dst = QT[:, m, tsl] if m < 4 else KT[:, m - 4, tsl]
            nc.vector.tensor_copy(dst, qk_ps)
        # V rows for this chunk
        for tt in range(4):
            v_ps = ps.tile([128, 512], F32, tag="mm")
            for ci in range(8):
                nc.tensor.matmul(
                    v_ps[:],
                    xnc[:, ci, tt * 128 : (tt + 1) * 128],
                    wv_sb[:, ci, :],
                    start=(ci == 0), stop=(ci == 7),
                )
            nc.vector.tensor_copy(V[:, tcx * 4 + tt, :], v_ps)

    # =============== Phase B: attention per (head-pair, q-chunk) ===========
    for hp in range(4):
        for qc in range(4):
            nkt = 4 * qc + 4
            qsl = slice(qc * 512, (qc + 1) * 512)
            yav = psacc.tile([128, 512], F32, tag="acc_ps")
            acc_a = workB.tile([128, 512], BF, tag="acc_a")
            acc_b = workB.tile([128, 512], BF, tag="acc_b")
            for kt in range(nkt):
                ksl = slice(kt * 128, (kt + 1) * 128)
                sa = ps.tile([128, 512], F32, tag="mm")
                sb = ps.tile([128, 512], F32, tag="mm")
                nc.tensor.matmul(
                    sa[:], KT[0:64, hp, ksl], QT[0:64, hp, qsl],
                    start=True, stop=True,
                )
                nc.tensor.matmul(
                    sb[:], KT[64:128, hp, ksl], QT[64:128, hp, qsl],
                    start=True, stop=True,
                )
                ea = workB.tile([128, 512], BF, tag="ea")
                eb = workB.tile([128, 512], BF, tag="eb")
                nc.scalar.activation(ea, sa, AF.Exp)
                nc.scalar.activation(eb, sb, AF.Exp)
                if kt >= nkt - 4:
                    j = kt - (nkt - 4)
                    for e in (ea, eb):
                        nc.gpsimd.affine_select(
                            out=e, in_=e,
                            compare_op=mybir.AluOpType.is_ge,
                            fill=0.0, base=-128 * j,
                            channel_multiplier=-1, pattern=[[1, 512]],
                        )
                if kt == 0:
                    nc.vector.tensor_copy(acc_a, ea)
                    nc.vector.tensor_copy(acc_b, eb)
                else:
                    nc.vector.tensor_add(acc_a, acc_a, ea)
                    nc.vector.tensor_add(acc_b, acc_b, eb)
                nc.tensor.matmul(
                    yav[0:64, :], V[:, kt, 2 * hp * 64 : (2 * hp + 1) * 64], ea[:],
                    start=(kt == 0), stop=(kt == nkt - 1),
                    tile_position=(0, 0),
                )
                nc.tensor.matmul(
                    yav[64:128, :],
                    V[:, kt, (2 * hp + 1) * 64 : (2 * hp + 2) * 64], eb[:],
                    start=(kt == 0), stop=(kt == nkt - 1),
                    tile_position=(0, 64),
                )
            den = psacc.tile([128, 512], F32, tag="acc_ps")
            nc.tensor.matmul(
                den[0:64, :], ones64[:], acc_a[:],
                start=True, stop=True, tile_position=(0, 0),
            )
            nc.tensor.matmul(
                den[64:128, :], ones64[:], acc_b[:],
                start=True, stop=True, tile_position=(0, 64),
            )
            rden = workB.tile([128, 512], F32, tag="rden")
            nc.vector.reciprocal(rden, den)
            nc.vector.tensor_mul(yT[:, hp, qsl], yav, rden)
    workB.release()
    workA.release()
    pool_ab.release()

    # =============== Phase C: pairwise head exchange =======================
    # AllGather the per-head attention outputs within each core pair, then
    # each core keeps its token half (dynamic column offset from `tokoff`).
    inb = dram.tile([512, T], BF, tag="inb")
    outb = dram.tile([1024, T], BF, tag="outb")
    for hp in range(4):
        nc.sync.dma_start(out=inb[hp * 128 : (hp + 1) * 128, :], in_=yT[:, hp, :])
    if collective:
        nc.gpsimd.collective_compute(
            "AllGather",
            mybir.AluOpType.bypass,
            replica_groups=[[0, 1], [2, 3], [4, 5], [6, 7]],
            ins=[inb[:].opt()],
            outs=[outb[:].opt()],
        )
    else:
        nc.sync.dma_start(out=outb[:512, :], in_=inb[:])
        nc.sync.dma_start(out=outb[512:, :], in_=inb[:])
    pool_y.release()

    tok_reg = nc.sync.alloc_register("tokoff_reg")
    nc.sync.reg_load(tok_reg, _g_tokoff[0:1, 0:1])
    tok0 = nc.sync.snap(tok_reg, donate=True, min_val=0, max_val=TQ)

    pool_d = tc.alloc_tile_pool(name="pool_d", bufs=1)
    pool_e = tc.alloc_tile_pool(name="pool_e", bufs=1)
    workD = tc.alloc_tile_pool(name="workD", bufs=2)
    yTf = pool_d.tile([128, 8, 1024], BF, tag="yTf")
    for r in range(8):
        nc.sync.dma_start(
            out=yTf[:, r, :],
            in_=outb[r * 128 : (r + 1) * 128, bass.ds(tok0, TQ)],
        )

    # =============== Phase D: proj + residual + rmsnorm2 + transpose =======
    wp_sb = pool_d.tile([128, 8, 1024], BF, tag="wp_sb")
    nc.sync.dma_start(out=wp_sb, in_=wp.rearrange("(ci p) m -> p ci m", p=128))
    xqr = xq.rearrange("(tt p) c -> p tt c", p=128)

    zd = dram.tile([TQ, C], F32, tag="zd")
    xn2 = pool_d.tile([128, 8, 1024], BF, tag="xn2")
    for tt in range(8):
        xq_tt = workD.tile([128, 1024], F32, tag="xq_tt")
        nc.sync.dma_start(out=xq_tt, in_=xqr[:, tt, :])
        z_tt = workD.tile([128, 1024], F32, tag="z_tt")
        for coh in range(2):
            csl = slice(coh * 512, (coh + 1) * 512)
            pp = psacc.tile([128, 512], F32, tag="acc_ps")
            for ci in range(8):
                nc.tensor.matmul(
                    pp[:],
                    yTf[:, ci, tt * 128 : (tt + 1) * 128],
                    wp_sb[:, ci, csl],
                    start=(ci == 0), stop=(ci == 7),
                )
            nc.vector.tensor_add(z_tt[:, csl], xq_tt[:, csl], pp)
        nc.sync.dma_start(out=zd[tt * 128 : (tt + 1) * 128, :], in_=z_tt)
        sq2 = workD.tile([128, 1024], F32, tag="sq2")
        ss2 = workD.tile([128, 1], F32, tag="ss2")
        nc.scalar.activation(sq2, z_tt, AF.Square, accum_out=ss2)
        s2 = workD.tile([128, 1], F32, tag="s2")
        nc.scalar.activation(s2, ss2, AF.Sqrt, bias=eps_sb[:], scale=1.0 / C)
        r2 = workD.tile([128, 1], F32, tag="r2")
        nc.vector.reciprocal(r2, s2)
        nc.vector.tensor_scalar_mul(xn2[:, tt, :], z_tt, r2)

    xn2T = pool_e.tile([128, 8, 1024], BF, tag="xn2T")
    for ci in range(8):
        for tt in range(8):
            tp = ps.tile([128, 128], BF, tag="mm")
            nc.tensor.transpose(tp[:], xn2[:, tt, ci * 128 : (ci + 1) * 128], ident[:])
            nc.any.tensor_copy(xn2T[:, ci, tt * 128 : (tt + 1) * 128], tp)
    workD.release()
    workE = tc.alloc_tile_pool(name="workE", bufs=3)
    poolH = tc.alloc_tile_pool(name="poolH", bufs=10)
    poolW2 = tc.alloc_tile_pool(name="poolW2", bufs=2)

    # =============== Phase E: MLP (fc1/fc2 group-pipelined, no staging) ====
    out_acc = pool_e.tile([128, 8, 1024], F32, tag="out_acc")
    wf2r = wf2.rearrange("(m p) co -> m p co", p=128)
    outr = out.rearrange("(tt p) c -> p tt c", p=128)
    for mg in range(4):
        hrows = []
        wf2g = poolW2.tile([128, 8, 1024], BF, tag="wf2g")
        for j in range(8):
            m = mg * 8 + j
            nc.sync.dma_start(out=wf2g[:, j, :], in_=wf2r[m])
            w1s = workE.tile([128, 8, 128], BF, tag="w1s")
            nc.sync.dma_start(out=w1s, in_=wf1[m].rearrange("ci p q -> p ci q"))
            hrow = poolH.tile([128, 1024], BF, tag="hrow")
            for tch in range(2):
                tsl = slice(tch * 512, (tch + 1) * 512)
                h_ps = ps.tile([128, 512], F32, tag="mm")
                for ci in range(8):
                    nc.tensor.matmul(
                        h_ps[:], w1s[:, ci, :], xn2T[:, ci, tsl],
                        start=(ci == 0), stop=(ci == 7),
                    )
                hrelu = workE.tile([128, 512], BF, tag="hrelu")
                nc.scalar.activation(hrelu, h_ps, AF.Relu)
                nc.vector.tensor_mul(hrow[:, tsl], hrelu, hrelu)
            hrows.append(hrow)
        for tt in range(8):
            for coh in range(2):
                csl = slice(coh * 512, (coh + 1) * 512)
                mp = psacc.tile([128, 512], F32, tag="acc_ps")
                for j in range(8):
                    nc.tensor.matmul(
                        mp[:],
                        hrows[j][:, tt * 128 : (tt + 1) * 128],
                        wf2g[:, j, csl],
                        start=(j == 0), stop=(j == 7),
                    )
                if mg == 0:
                    nc.vector.tensor_copy(out_acc[:, tt, csl], mp)
                else:
                    nc.vector.tensor_add(out_acc[:, tt, csl], out_acc[:, tt, csl], mp)
    for tt in range(8):
        for coh in range(2):
            csl = slice(coh * 512, (coh + 1) * 512)
            zr = workE.tile([128, 512], F32, tag="zr")
            nc.sync.dma_start(out=zr, in_=zd[tt * 128 : (tt + 1) * 128, csl])
            ot = workE.tile([128, 512], F32, tag="ot")
            nc.vector.tensor_add(ot, zr, out_acc[:, tt, csl])
            nc.sync.dma_start(out=outr[:, tt, csl], in_=ot)
    poolW2.release()
    poolH.release()
    workE.release()
    pool_e.release()
    pool_d.release()

    ctx.close()


def _prep_inputs(x, w_qkv, w_proj, w_fc1, w_fc2, scale1, scale2):
    """Host-side sharding: returns in_maps for the 8 cores."""
    bf = ml_dtypes.bfloat16
    Wq = (w_qkv[:C] * scale1[None, :]) * (1.0 / np.sqrt(D))
    Wk = w_qkv[C : 2 * C] * scale1[None, :]
    Wv = w_qkv[2 * C :] * scale1[None, :]
    wp_np = np.ascontiguousarray(w_proj.T).astype(bf)
    wf1T = (w_fc1 * scale2[None, :]).T  # [C, 4C]
    wf1_np = np.ascontiguousarray(
        wf1T.reshape(8, 128, 32, 128).transpose(2, 0, 1, 3)
    ).astype(bf)
    wf2_np = np.ascontiguousarray(w_fc2.T).astype(bf)

    in_maps = []
    for core in range(NCORES):
        b, half = divmod(core, 2)
        heads = [8 * half + j for j in range(HPC)]
        qk_rows = []
        for i in range(4):
            h0, h1 = heads[2 * i], heads[2 * i + 1]
            qk_rows.append(Wq[64 * h0 : 64 * h0 + 64])
            qk_rows.append(Wq[64 * h1 : 64 * h1 + 64])
        for i in range(4):
            h0, h1 = heads[2 * i], heads[2 * i + 1]
            qk_rows.append(Wk[64 * h0 : 64 * h0 + 64])
            qk_rows.append(Wk[64 * h1 : 64 * h1 + 64])
        wqk_np = np.ascontiguousarray(np.concatenate(qk_rows, axis=0).T).astype(bf)
        v_rows = np.concatenate(
            [Wv[64 * h : 64 * h + 64] for h in heads], axis=0
        )
        wv_np = np.ascontiguousarray(v_rows.T).astype(bf)
        in_maps.append(
            {
                "xT": np.ascontiguousarray(x[b].T).astype(np.float32),
                "xq": np.ascontiguousarray(
                    x[b, half * TQ : (half + 1) * TQ]
                ).astype(np.float32),
                "wqk": wqk_np,
                "wv": wv_np,
                "wp": wp_np,
                "wf1": wf1_np,
                "wf2": wf2_np,
                "tokoff": np.array([[half * TQ]], dtype=np.uint32),
            }
        )
    return in_maps


def get_nc(collective: bool = True):
    key = ("nc", collective)
    if key not in _cache:
        _cache[key] = _build(collective=collective)
    return _cache[key]


def kernel(x, w_qkv, w_proj, w_fc1, w_fc2, scale1, scale2):
    x = np.asarray(x, dtype=np.float32)
    w_qkv = np.asarray(w_qkv, dtype=np.float32)
    w_proj = np.asarray(w_proj, dtype=np.float32)
    w_fc1 = np.asarray(w_fc1, dtype=np.float32)
    w_fc2 = np.asarray(w_fc2, dtype=np.float32)
    scale1 = np.asarray(scale1, dtype=np.float32)
    scale2 = np.asarray(scale2, dtype=np.float32)

    nc = get_nc(collective=True)
    in_maps = _prep_inputs(x, w_qkv, w_proj, w_fc1, w_fc2, scale1, scale2)
    res = bass_utils.run_bass_kernel_spmd(
        nc, in_maps, core_ids=list(range(NCORES)), trace=False
    )
    out = np.empty((B, T, C), dtype=np.float32)
    for core in range(NCORES):
        b, half = divmod(core, 2)
        out[b, half * TQ : (half + 1) * TQ] = res.results[core]["out"]
    return out
